# revision 10
# baseline (speedup 1.0000x reference)
"""Trainium2 Bass kernel for nn_DecoderCell (B=128,N=512,C=4,T=128,D=128,H=8).

Strategy: pure data-parallel over batch B across 8 NeuronCores (16 b/core).
Per batch, attention scores are computed transposed ([n, q] layout, q=(t,c))
with per-head K=16 matmuls packed 4-at-a-time onto PE row groups; the boolean
mask enters PSUM via identity-matmul adds; softmax runs unnormalized
(exp on ScalarE, denominators via an augmented-V ones column), and the final
log-softmax uses ACT accum_out row sums plus a DVE bit-twiddle ln.
"""
import numpy as np
import ml_dtypes

D = 128
N = 512
C = 4
T = 128
Q = T * C          # 512 queries per batch, q = t*C + c
H = 8
DH = 16
NB = 16            # batches per core
NCORES = 8
NEG = -1e9         # reference mask value
NEGT = -1e8        # written into tanh slots; ×10 → -1e9

# ln(m) on [1,2), power-series coeffs (deg 7, max err 5.6e-7)
LN_COEF = [
    -2.242481818575902, 4.911042808776086, -5.126667255647402,
    3.932633388234101, -2.0202020938525127, 0.6590148821953554,
    -0.12345843186141042, 0.010119082927599773,
]
LN2 = 0.6931471805599453

HA = [0, 1, 2, 3]
HB = [4, 5, 6, 7]


def _perm_cols(W, heads):
    """Columns of W[*,128] so head g sits at cols 32g..32g+15, zeros after."""
    out = np.zeros_like(W)
    for g, h in enumerate(heads):
        out[:, 32 * g:32 * g + 16] = W[:, 16 * h:16 * h + 16]
    return out


def _perm_rows(W, heads):
    out = np.zeros_like(W)
    for g, h in enumerate(heads):
        out[32 * g:32 * g + 16, :] = W[16 * h:16 * h + 16, :]
    return out


def _host_prep(inputs):
    """Full-input numpy prep -> (shared weight dict, per-core input dicts)."""
    ne = np.ascontiguousarray(inputs["node_embeddings"], np.float32)
    ge = np.ascontiguousarray(inputs["graph_embedding"], np.float32)
    sc = np.ascontiguousarray(inputs["step_context"], np.float32)
    mask = np.asarray(inputs["mask"])
    B = ne.shape[0]

    net = ne.transpose(0, 2, 1)                                   # [B,D,N]
    scq = sc[:, :, :, 0, :].transpose(1, 3, 0, 2).reshape(B, D + 1, Q)
    scm, scl = scq[:, :D, :], scq[:, D:, :]                       # [B,128,Q],[B,1,Q]
    m4 = mask[:, :, :, 0, :]                                      # [T,B,C,N] bool
    mkt = (m4.transpose(1, 3, 0, 2).reshape(B, N, Q).astype(np.float32)
           * np.float32(NEG)).astype(ml_dtypes.bfloat16)          # [B,N,Q]
    mknat = m4.transpose(1, 0, 2, 3).reshape(B, Q, N).astype(np.uint8)
    gt = np.ascontiguousarray(ge.T)                               # [D,B]

    s = np.float32(1.0 / np.sqrt(DH))
    Wk1 = np.asarray(inputs["Wk1"], np.float32)
    Wqs = np.asarray(inputs["Wq_step"], np.float32) * s
    Wqf = np.asarray(inputs["Wq_fixed"], np.float32) * s
    wqsa, wqsb = _perm_cols(Wqs, HA), _perm_cols(Wqs, HB)
    weights = {
        "wk1a": _perm_cols(Wk1, HA), "wk1b": _perm_cols(Wk1, HB),
        "wqsam": wqsa[:D], "wqsal": wqsa[D:],
        "wqsbm": wqsb[:D], "wqsbl": wqsb[D:],
        "wqfa": _perm_cols(Wqf, HA), "wqfb": _perm_cols(Wqf, HB),
        "wv": np.asarray(inputs["Wv"], np.float32),
        "wk2s": np.asarray(inputs["Wk2"], np.float32) / np.float32(np.sqrt(D)),
        "wouta": _perm_rows(np.asarray(inputs["Wout"], np.float32), HA),
        "woutb": _perm_rows(np.asarray(inputs["Wout"], np.float32), HB),
        "i128b": np.eye(128, dtype=ml_dtypes.bfloat16),
        "ident": np.eye(128, dtype=np.float32),
        # e4[g, 32g+r] = 1 for r<16 (normalization broadcast)
        "e4": np.stack([
            np.concatenate([np.zeros(32 * g, np.float32),
                            np.ones(16, np.float32),
                            np.zeros(128 - 32 * g - 16, np.float32)])
            for g in range(4)]),
        # g16[32g+16, g] = 1 (Z row gather)
        "g16": np.stack([
            (np.arange(128) == 32 * g + 16).astype(np.float32)
            for g in range(4)], axis=1),
        # p432[tp, 4tp+c] = 1 (lnZ partition broadcast)
        "p432": np.stack([
            ((np.arange(128) // 4) == tp).astype(np.float32)
            for tp in range(32)]),
    }

    core_ins = []
    for ci in range(NCORES):
        b0 = ci * NB
        sl = slice(b0, b0 + NB)
        m = dict(weights)
        m.update({
            "net": np.ascontiguousarray(net[sl]),
            "scm": np.ascontiguousarray(scm[sl]),
            "scl": np.ascontiguousarray(scl[sl]),
            "mkt": np.ascontiguousarray(mkt[sl]),
            "mknat": np.ascontiguousarray(mknat[sl]),
            "gt": np.ascontiguousarray(gt[:, sl]),
        })
        core_ins.append(m)
    return core_ins


def build_kernel(nb=NB):
    import concourse.bacc as bacc
    import concourse.mybir as mybir
    import concourse.tile as tile

    dt = mybir.dt
    f32, bf16, u8, i32 = dt.float32, dt.bfloat16, dt.uint8, dt.int32
    AF = mybir.ActivationFunctionType
    OP = mybir.AluOpType

    nc = bacc.Bacc("TRN2", target_bir_lowering=False, debug=False,
                   num_devices=NCORES)

    din = {}
    def dram(name, shape, dtype, kind="ExternalInput"):
        din[name] = nc.dram_tensor(name, shape, dtype, kind=kind)
        return din[name]

    net = dram("net", [nb, D, N], f32)
    scm = dram("scm", [nb, D, Q], f32)
    scl = dram("scl", [nb, 1, Q], f32)
    mkt = dram("mkt", [nb, N, Q], bf16)
    mknat = dram("mknat", [nb, Q, N], u8)
    gt = dram("gt", [D, nb], f32)
    for w in ("wk1a", "wk1b", "wqsam", "wqsbm", "wqfa", "wqfb", "wv", "wk2s",
              "wouta", "woutb", "ident"):
        dram(w, [128, 128], f32)
    dram("i128b", [128, 128], bf16)
    dram("wqsal", [1, 128], f32)
    dram("wqsbl", [1, 128], f32)
    dram("e4", [4, 128], f32)
    dram("g16", [128, 4], f32)
    dram("p432", [32, 128], f32)
    # device layout [q'=(t', c), b, i, n]; host reassembles t = 32*i + t'
    out = dram("out", [128, nb, 4, N], f32, kind="ExternalOutput")

    with tile.TileContext(nc) as tc:
        from contextlib import ExitStack
        with ExitStack() as ctx:
            wp = ctx.enter_context(tc.tile_pool(name="wp", bufs=1))
            io = ctx.enter_context(tc.tile_pool(name="io", bufs=2))
            wk = ctx.enter_context(tc.tile_pool(name="wk", bufs=2))
            big = ctx.enter_context(tc.tile_pool(name="big", bufs=2))
            sm = ctx.enter_context(tc.tile_pool(name="sm", bufs=2))
            pbig = ctx.enter_context(tc.tile_pool(name="pbig", bufs=2, space="PSUM"))
            pu = ctx.enter_context(tc.tile_pool(name="pu", bufs=1, space="PSUM"))
            pproj = ctx.enter_context(tc.tile_pool(name="pproj", bufs=1, space="PSUM"))
            ptiny = ctx.enter_context(tc.tile_pool(name="ptiny", bufs=2, space="PSUM"))

            # --- static weights/constants to SBUF ---
            W = {}
            for wn in ("wk1a", "wk1b", "wqsam", "wqsbm", "wqfa", "wqfb", "wv",
                       "wk2s", "wouta", "woutb", "ident", "i128b", "e4", "g16",
                       "p432", "wqsal", "wqsbl", "gt"):
                t = wp.tile(list(din[wn].shape), din[wn].dtype, tag=f"w_{wn}")
                nc.sync.dma_start(out=t, in_=din[wn][:, :])
                W[wn] = t
            negt = wp.tile([128, C, N], f32, tag="negt")
            nc.gpsimd.memset(negt, NEGT)

            for b in range(nb):
                # ---------- DMA loads ----------
                net_t = io.tile([D, N], f32, tag="net")
                nc.sync.dma_start(out=net_t, in_=net[b])
                scm_t = io.tile([D, Q], f32, tag="scm")
                nc.sync.dma_start(out=scm_t, in_=scm[b])
                scl_t = io.tile([1, Q], f32, tag="scl")
                nc.sync.dma_start(out=scl_t, in_=scl[b])
                # mkt[b] [N, Q] -> [128, 4(j), Q]
                mkt_t = io.tile([128, 4, Q], bf16, tag="mkt")
                nc.sync.dma_start(
                    out=mkt_t, in_=mkt[b].rearrange("(j p) q -> p j q", p=128))
                # mknat[b] [Q, N] -> [128, 4(i), N]
                mknat_t = io.tile([128, 4, N], u8, tag="mknat")
                nc.sync.dma_start(
                    out=mknat_t, in_=mknat[b].rearrange("(i p) n -> p i n", p=128))

                # ---------- projections ----------
                def proj_to_sbuf(wtile, rhs, tag, scale_dst=None):
                    ps = pproj.tile([128, N], f32, tag="proj")
                    nc.tensor.matmul(ps, lhsT=wtile, rhs=rhs)
                    sb = wk.tile([128, N], f32, tag=tag)
                    nc.vector.tensor_copy(sb, ps)
                    return sb

                k1ta = proj_to_sbuf(W["wk1a"], net_t, "k1ta")
                k1tb = proj_to_sbuf(W["wk1b"], net_t, "k1tb")
                k2t = proj_to_sbuf(W["wk2s"], net_t, "k2t")

                # V natural per n-chunk, packed into V_aug [128, 4(j), 128]
                vauga = wk.tile([128, 4, 128], f32, tag="vauga")
                vaugb = wk.tile([128, 4, 128], f32, tag="vaugb")
                for va in (vauga, vaugb):
                    nc.gpsimd.memset(va, 0.0)
                    nc.gpsimd.memset(
                        va.rearrange("p j (g r) -> p j g r", g=4)[:, :, :, 16:17],
                        1.0)
                for j in range(4):
                    pv = pproj.tile([128, 128], f32, tag="proj")
                    nc.tensor.matmul(
                        pv[:, :128], lhsT=net_t[:, 128 * j:128 * (j + 1)],
                        rhs=W["wv"])
                    for va, c0 in ((vauga, 0), (vaugb, 64)):
                        nc.vector.tensor_copy(
                            va[:, j, :].rearrange("p (g r) -> p g r", g=4)[:, :, 0:16],
                            pv[:, c0:c0 + 64].rearrange("p (g r) -> p g r", g=4))

                # ---------- Q1T (passes A and B) ----------
                def q1t(wm, wl, wf, tag):
                    ps = pproj.tile([128, Q], f32, tag="proj")
                    nc.tensor.matmul(ps, lhsT=wm, rhs=scm_t, start=True, stop=False)
                    nc.tensor.matmul(ps, lhsT=wl, rhs=scl_t, start=False, stop=False)
                    gt_bcast = W["gt"][:, b:b + 1].broadcast_to([128, Q])
                    nc.tensor.matmul(ps, lhsT=wf, rhs=gt_bcast,
                                     start=False, stop=True)
                    sb = wk.tile([128, Q], f32, tag=tag)
                    nc.vector.tensor_copy(sb, ps)
                    return sb

                q1ta = q1t(W["wqsam"], W["wqsal"], W["wqfa"], "q1ta")
                q1tb = q1t(W["wqsbm"], W["wqsbl"], W["wqfb"], "q1tb")

                # ---------- attention passes ----------
                zsb = sm.tile([4, 1024], f32, tag="zsb")
                usb = {}
                for pi, (k1t, q1t_sb) in enumerate(((k1ta, q1ta), (k1tb, q1tb))):
                    psu = pu.tile([128, Q], f32, tag="u")
                    for j in range(4):
                        for half in range(2):
                            pss = pbig.tile([128, 2, Q], f32, tag="bigp")
                            es = big.tile([128, 2, Q], f32, tag="expS")
                            for gg in range(2):
                                g = 2 * half + gg
                                sl = slice(32 * g, 32 * g + 16)
                                nc.tensor.matmul(
                                    pss[:, gg, :],
                                    lhsT=k1t[sl, 128 * j:128 * (j + 1)],
                                    rhs=q1t_sb[sl, :], start=True, stop=False,
                                    tile_position=(32 * g, 0))
                                nc.tensor.matmul(
                                    pss[:, gg, :], lhsT=W["i128b"],
                                    rhs=mkt_t[:, j, :], start=False, stop=True)
                            nc.scalar.activation(es, pss, AF.Exp)
                            for gg in range(2):
                                g = 2 * half + gg
                                nc.tensor.matmul(
                                    psu[32 * g:32 * g + 32, :],
                                    lhsT=vauga[:, j, 32 * g:32 * g + 32]
                                    if pi == 0 else vaugb[:, j, 32 * g:32 * g + 32],
                                    rhs=es[:, gg, :],
                                    start=(j == 0), stop=(j == 3),
                                    tile_position=(0, 32 * g))
                    # copy U to sbuf; gather Z rows; normalize later
                    u_sb = wk.tile([128, Q], f32, tag="usb")
                    nc.vector.tensor_copy(u_sb, psu)
                    usb[pi] = u_sb
                    pz = ptiny.tile([4, Q], f32, tag="tiny")
                    nc.tensor.matmul(pz, lhsT=W["g16"], rhs=u_sb)
                    nc.vector.tensor_copy(zsb[:, Q * pi:Q * (pi + 1)], pz)

                rinv = sm.tile([4, 1024], f32, tag="rinv")
                nc.vector.reciprocal_approx_fast(out=rinv, in_=zsb)

                un = {}
                for pi in range(2):
                    pbc = pproj.tile([128, Q], f32, tag="proj")
                    nc.tensor.matmul(pbc, lhsT=W["e4"],
                                     rhs=rinv[:, Q * pi:Q * (pi + 1)])
                    u_n = wk.tile([128, Q], f32, tag=f"un{pi}")
                    nc.vector.tensor_tensor(u_n, usb[pi], pbc, OP.mult)
                    un[pi] = u_n

                # ---------- Q2 and logits ----------
                pq2 = pproj.tile([128, Q], f32, tag="proj")
                nc.tensor.matmul(pq2, lhsT=W["wouta"], rhs=un[0],
                                 start=True, stop=False)
                nc.tensor.matmul(pq2, lhsT=W["woutb"], rhs=un[1],
                                 start=False, stop=True)
                q2t = wk.tile([128, Q], f32, tag="q2t")
                nc.vector.tensor_copy(q2t, pq2)

                tanh_sb = big.tile([128, C, N], f32, tag="tanh")
                for ii in range(2):
                    pl = pbig.tile([128, 2, N], f32, tag="bigp")
                    for i2 in range(2):
                        i = 2 * ii + i2
                        nc.tensor.matmul(
                            pl[:, i2, :],
                            lhsT=q2t[:, 128 * i:128 * (i + 1)], rhs=k2t)
                    nc.scalar.activation(
                        tanh_sb[:, 2 * ii:2 * ii + 2, :], pl, AF.Tanh)

                nc.vector.copy_predicated(tanh_sb, mknat_t, negt)

                # ---------- final log-softmax ----------
                sacc = sm.tile([128, 4], f32, tag="sacc")
                scratch = sm.tile([128, N], f32, tag="scratch")
                for i in range(4):
                    nc.scalar.activation(scratch, tanh_sb[:, i, :], AF.Exp,
                                         scale=10.0, accum_out=sacc[:, i:i + 1])
                # S [128(q'),4(i)] -> [4, 128] -> Z2 [4, 32]
                pst = ptiny.tile([4, 128], f32, tag="tiny")
                nc.tensor.transpose(pst, sacc, W["ident"])
                ssb = sm.tile([4, 128], f32, tag="ssb")
                nc.vector.tensor_copy(ssb, pst)
                z2 = sm.tile([4, 32], f32, tag="z2")
                nc.vector.tensor_reduce(
                    z2, ssb.rearrange("p (t c) -> p t c", c=4),
                    axis=mybir.AxisListType.X, op=OP.add)
                # ln via exponent bits + poly
                zi = z2.bitcast(i32)
                ei = sm.tile([4, 32], i32, tag="ei")
                nc.vector.tensor_scalar(ei, zi, 23, None,
                                        OP.logical_shift_right)
                ef = sm.tile([4, 32], f32, tag="ef")
                nc.vector.tensor_copy(ef, ei)
                mi = sm.tile([4, 32], i32, tag="mi")
                nc.vector.tensor_scalar(mi, zi, 0x7FFFFF, 0x3F800000,
                                        OP.bitwise_and, OP.bitwise_or)
                mf = mi.bitcast(f32)
                acc = sm.tile([4, 32], f32, tag="lnacc")
                nc.vector.tensor_scalar(acc, mf, LN_COEF[7], LN_COEF[6],
                                        OP.mult, OP.add)
                for k in range(5, -1, -1):
                    nc.vector.tensor_tensor(acc, acc, mf, OP.mult)
                    nc.vector.tensor_scalar_add(acc, acc, LN_COEF[k])
                # lnZ2 = acc + (ef - 127)*LN2
                nc.vector.tensor_scalar(ef, ef, LN2, -127.0 * LN2,
                                        OP.mult, OP.add)
                nc.vector.tensor_tensor(acc, acc, ef, OP.add)
                # transpose [4,32] -> [32,4]; broadcast to [128,4]
                pzt = ptiny.tile([32, 4], f32, tag="tiny")
                nc.tensor.transpose(pzt, acc, W["ident"][:4, :4])
                lzt = sm.tile([32, 4], f32, tag="lzt")
                nc.vector.tensor_copy(lzt, pzt)
                pbias = ptiny.tile([128, 4], f32, tag="tiny")
                nc.tensor.matmul(pbias, lhsT=W["p432"], rhs=lzt)
                bias = sm.tile([128, 4], f32, tag="bias")
                nc.vector.tensor_copy(bias, pbias)

                out_sb = big.tile([128, C, N], f32, tag="outsb")
                for i in range(4):
                    nc.vector.tensor_scalar(
                        out_sb[:, i, :], tanh_sb[:, i, :], 10.0,
                        bias[:, i:i + 1], OP.mult, OP.subtract)
                nc.sync.dma_start(out=out[:, b, :, :], in_=out_sb)

    nc.compile()
    return nc


_CACHED = None


def _get_nc():
    global _CACHED
    if _CACHED is None:
        _CACHED = build_kernel()
    return _CACHED


def kernel(**inputs):
    from concourse.bass_utils import run_bass_kernel_spmd

    core_ins = _host_prep(inputs)
    nc = _get_nc()
    res = run_bass_kernel_spmd(nc, core_ins, core_ids=list(range(NCORES)))
    outs = [_unscramble(r["out"]) for r in res.results]   # each [T, NB, 2048]
    return np.concatenate(outs, axis=1)                   # [T, B, 2048]


def _unscramble(dev):
    """Device [128 q'=(t',c), nb, 4 i, 512 n] -> [T, nb, C*N] with t=32i+t'."""
    nb = dev.shape[1]
    return (dev.reshape(32, C, nb, 4, N)
            .transpose(3, 0, 2, 1, 4)
            .reshape(T, nb, C * N))


# revision 12
# speedup vs baseline: 2.2575x; 2.2575x over previous
"""Trainium2 Bass kernel for nn_DecoderCell (B=128,N=512,C=4,T=128,D=128,H=8).

Strategy: pure data-parallel over batch B across 8 NeuronCores (16 b/core).
Per batch, attention scores are computed transposed ([n, q] layout, q=(t,c))
with per-head K=16 matmuls packed 4-at-a-time onto PE row groups; the boolean
mask enters PSUM via identity-matmul adds; softmax runs unnormalized
(exp on ScalarE, denominators via an augmented-V ones column), and the final
log-softmax uses ACT accum_out row sums plus a DVE bit-twiddle ln.
Matmul operands are bf16 (PSUM accumulation in f32) for single-pass PE speed.
"""
import numpy as np
import ml_dtypes

D = 128
N = 512
C = 4
T = 128
Q = T * C          # 512 queries per batch, q = t*C + c
H = 8
DH = 16
NB = 16            # batches per core
NCORES = 8
NEG = -1e9         # reference mask value
NEGT = -1e8        # written into tanh slots; ×10 → -1e9

# ln(m) on [1,2), power-series coeffs (deg 7, max err 5.6e-7)
LN_COEF = [
    -2.242481818575902, 4.911042808776086, -5.126667255647402,
    3.932633388234101, -2.0202020938525127, 0.6590148821953554,
    -0.12345843186141042, 0.010119082927599773,
]
LN2 = 0.6931471805599453

HA = [0, 1, 2, 3]
HB = [4, 5, 6, 7]
BF = ml_dtypes.bfloat16


def _perm_cols(W, heads):
    """Columns of W[*,128] so head g sits at cols 32g..32g+15, zeros after."""
    out = np.zeros_like(W)
    for g, h in enumerate(heads):
        out[:, 32 * g:32 * g + 16] = W[:, 16 * h:16 * h + 16]
    return out


def _perm_rows(W, heads):
    out = np.zeros_like(W)
    for g, h in enumerate(heads):
        out[32 * g:32 * g + 16, :] = W[16 * h:16 * h + 16, :]
    return out


def _host_prep(inputs):
    """Full-input numpy prep -> per-core input dicts."""
    ne = np.ascontiguousarray(inputs["node_embeddings"], np.float32)
    ge = np.ascontiguousarray(inputs["graph_embedding"], np.float32)
    sc = np.ascontiguousarray(inputs["step_context"], np.float32)
    mask = np.asarray(inputs["mask"])
    B = ne.shape[0]

    net = ne.transpose(0, 2, 1).astype(BF)                        # [B,D,N]
    scq = sc[:, :, :, 0, :].transpose(1, 3, 0, 2).reshape(B, D + 1, Q)
    scm = scq[:, :D, :].astype(BF)
    scl = scq[:, D:, :].astype(BF)
    m4 = mask[:, :, :, 0, :]                                      # [T,B,C,N] bool
    mkt = (m4.transpose(1, 3, 0, 2).reshape(B, N, Q).astype(np.float32)
           * np.float32(NEG)).astype(BF)                          # [B,N,Q]
    mknat = m4.transpose(1, 0, 2, 3).reshape(B, Q, N).astype(np.uint8)
    gt = np.ascontiguousarray(ge.T).astype(BF)                    # [D,B]

    s = np.float32(1.0 / np.sqrt(DH))
    Wk1 = np.asarray(inputs["Wk1"], np.float32)
    Wqs = np.asarray(inputs["Wq_step"], np.float32) * s
    Wqf = np.asarray(inputs["Wq_fixed"], np.float32) * s
    wqsa, wqsb = _perm_cols(Wqs, HA), _perm_cols(Wqs, HB)
    bfw = lambda x: np.ascontiguousarray(x).astype(BF)
    weights = {
        "wk1a": bfw(_perm_cols(Wk1, HA)), "wk1b": bfw(_perm_cols(Wk1, HB)),
        "wqsam": bfw(wqsa[:D]), "wqsal": bfw(wqsa[D:]),
        "wqsbm": bfw(wqsb[:D]), "wqsbl": bfw(wqsb[D:]),
        "wqfa": bfw(_perm_cols(Wqf, HA)), "wqfb": bfw(_perm_cols(Wqf, HB)),
        "wv": bfw(inputs["Wv"]),
        "wk2s": bfw(np.asarray(inputs["Wk2"], np.float32)
                    / np.float32(np.sqrt(D))),
        "wouta": bfw(_perm_rows(np.asarray(inputs["Wout"], np.float32), HA)),
        "woutb": bfw(_perm_rows(np.asarray(inputs["Wout"], np.float32), HB)),
        "i128b": np.eye(128, dtype=BF),
        "ident": np.eye(128, dtype=np.float32),
        # e4[g, 32g+r] = 1 for r<16 (normalization broadcast)
        "e4": np.stack([
            np.concatenate([np.zeros(32 * g, np.float32),
                            np.ones(16, np.float32),
                            np.zeros(128 - 32 * g - 16, np.float32)])
            for g in range(4)]).astype(BF),
        # g16[32g+16, g] = 1 (Z row gather)
        "g16": np.stack([
            (np.arange(128) == 32 * g + 16).astype(np.float32)
            for g in range(4)], axis=1).astype(BF),
        # p432[tp, 4tp+c] = 1 (lnZ partition broadcast)
        "p432": np.stack([
            ((np.arange(128) // 4) == tp).astype(np.float32)
            for tp in range(32)]),
    }

    core_ins = []
    for ci in range(NCORES):
        b0 = ci * NB
        sl = slice(b0, b0 + NB)
        m = dict(weights)
        m.update({
            "net": np.ascontiguousarray(net[sl]),
            "scm": np.ascontiguousarray(scm[sl]),
            "scl": np.ascontiguousarray(scl[sl]),
            "mkt": np.ascontiguousarray(mkt[sl]),
            "mknat": np.ascontiguousarray(mknat[sl]),
            "gt": np.ascontiguousarray(gt[:, sl]),
        })
        core_ins.append(m)
    return core_ins


def build_kernel(nb=NB):
    import concourse.bacc as bacc
    import concourse.mybir as mybir
    import concourse.tile as tile

    dt = mybir.dt
    f32, bf16, u8, i32 = dt.float32, dt.bfloat16, dt.uint8, dt.int32
    AF = mybir.ActivationFunctionType
    OP = mybir.AluOpType

    nc = bacc.Bacc("TRN2", target_bir_lowering=False, debug=False,
                   num_devices=NCORES)

    din = {}
    def dram(name, shape, dtype, kind="ExternalInput"):
        din[name] = nc.dram_tensor(name, shape, dtype, kind=kind)
        return din[name]

    net = dram("net", [nb, D, N], bf16)
    scm = dram("scm", [nb, D, Q], bf16)
    scl = dram("scl", [nb, 1, Q], bf16)
    mkt = dram("mkt", [nb, N, Q], bf16)
    mknat = dram("mknat", [nb, Q, N], u8)
    gt = dram("gt", [D, nb], bf16)
    for w in ("wk1a", "wk1b", "wqsam", "wqsbm", "wqfa", "wqfb", "wv", "wk2s",
              "wouta", "woutb", "i128b", "e4", "g16"):
        shape = [4, 128] if w == "e4" else ([128, 4] if w == "g16" else [128, 128])
        dram(w, shape, bf16)
    dram("ident", [128, 128], f32)
    dram("wqsal", [1, 128], bf16)
    dram("wqsbl", [1, 128], bf16)
    dram("p432", [32, 128], f32)
    # device layout [q'=(t', c), b, i, n]; host reassembles t = 32*i + t'
    out = dram("out", [128, nb, 4, N], f32, kind="ExternalOutput")

    with tile.TileContext(nc) as tc:
        from contextlib import ExitStack
        with ExitStack() as ctx:
            wp = ctx.enter_context(tc.tile_pool(name="wp", bufs=1))
            io = ctx.enter_context(tc.tile_pool(name="io", bufs=2))
            wk = ctx.enter_context(tc.tile_pool(name="wk", bufs=2))
            big = ctx.enter_context(tc.tile_pool(name="big", bufs=2))
            sm = ctx.enter_context(tc.tile_pool(name="sm", bufs=2))
            pbig = ctx.enter_context(tc.tile_pool(name="pbig", bufs=2, space="PSUM"))
            pu = ctx.enter_context(tc.tile_pool(name="pu", bufs=1, space="PSUM"))
            pproj = ctx.enter_context(tc.tile_pool(name="pproj", bufs=1, space="PSUM"))
            ptiny = ctx.enter_context(tc.tile_pool(name="ptiny", bufs=2, space="PSUM"))

            # --- static weights/constants to SBUF ---
            W = {}
            for wn in ("wk1a", "wk1b", "wqsam", "wqsbm", "wqfa", "wqfb", "wv",
                       "wk2s", "wouta", "woutb", "ident", "i128b", "e4", "g16",
                       "p432", "wqsal", "wqsbl", "gt"):
                t = wp.tile(list(din[wn].shape), din[wn].dtype, tag=f"w_{wn}")
                nc.sync.dma_start(out=t, in_=din[wn][:, :])
                W[wn] = t
            negt = wp.tile([128, C, N], f32, tag="negt")
            nc.gpsimd.memset(negt, NEGT)

            for b in range(nb):
                # ---------- DMA loads ----------
                net_t = io.tile([D, N], bf16, tag="net")
                nc.sync.dma_start(out=net_t, in_=net[b])
                scm_t = io.tile([D, Q], bf16, tag="scm")
                nc.sync.dma_start(out=scm_t, in_=scm[b])
                scl_t = io.tile([1, Q], bf16, tag="scl")
                nc.sync.dma_start(out=scl_t, in_=scl[b])
                # mkt[b] [N, Q] -> [128, 4(j), Q]
                mkt_t = io.tile([128, 4, Q], bf16, tag="mkt")
                nc.sync.dma_start(
                    out=mkt_t, in_=mkt[b].rearrange("(j p) q -> p j q", p=128))
                # mknat[b] [Q, N] -> [128, 4(i), N]
                mknat_t = io.tile([128, 4, N], u8, tag="mknat")
                nc.sync.dma_start(
                    out=mknat_t, in_=mknat[b].rearrange("(i p) n -> p i n", p=128))

                # ---------- projections ----------
                def proj_to_sbuf(wtile, rhs, tag):
                    ps = pproj.tile([128, N], f32, tag="proj")
                    nc.tensor.matmul(ps, lhsT=wtile, rhs=rhs)
                    sb = wk.tile([128, N], bf16, tag=tag)
                    nc.vector.tensor_copy(sb, ps)
                    return sb

                k1ta = proj_to_sbuf(W["wk1a"], net_t, "k1ta")
                k1tb = proj_to_sbuf(W["wk1b"], net_t, "k1tb")
                k2t = proj_to_sbuf(W["wk2s"], net_t, "k2t")

                # V natural per n-chunk, packed into V_aug [128, 4(j), 128]
                vauga = wk.tile([128, 4, 128], bf16, tag="vauga")
                vaugb = wk.tile([128, 4, 128], bf16, tag="vaugb")
                for va in (vauga, vaugb):
                    nc.gpsimd.memset(va, 0.0)
                    nc.gpsimd.memset(
                        va.rearrange("p j (g r) -> p j g r", g=4)[:, :, :, 16:17],
                        1.0)
                for j in range(4):
                    pv = pproj.tile([128, 128], f32, tag="proj")
                    nc.tensor.matmul(
                        pv[:, :128], lhsT=net_t[:, 128 * j:128 * (j + 1)],
                        rhs=W["wv"])
                    for va, c0 in ((vauga, 0), (vaugb, 64)):
                        nc.vector.tensor_copy(
                            va[:, j, :].rearrange("p (g r) -> p g r", g=4)[:, :, 0:16],
                            pv[:, c0:c0 + 64].rearrange("p (g r) -> p g r", g=4))

                # ---------- Q1T (passes A and B) ----------
                def q1t(wm, wl, wf, tag):
                    ps = pproj.tile([128, Q], f32, tag="proj")
                    nc.tensor.matmul(ps, lhsT=wm, rhs=scm_t, start=True, stop=False)
                    nc.tensor.matmul(ps, lhsT=wl, rhs=scl_t, start=False, stop=False)
                    gt_bcast = W["gt"][:, b:b + 1].broadcast_to([128, Q])
                    nc.tensor.matmul(ps, lhsT=wf, rhs=gt_bcast,
                                     start=False, stop=True)
                    sb = wk.tile([128, Q], bf16, tag=tag)
                    nc.vector.tensor_copy(sb, ps)
                    return sb

                q1ta = q1t(W["wqsam"], W["wqsal"], W["wqfa"], "q1ta")
                q1tb = q1t(W["wqsbm"], W["wqsbl"], W["wqfb"], "q1tb")

                # ---------- attention passes ----------
                zsb = sm.tile([4, 1024], f32, tag="zsb")
                usb = {}
                for pi, (k1t, q1t_sb) in enumerate(((k1ta, q1ta), (k1tb, q1tb))):
                    psu = pu.tile([128, Q], f32, tag="u")
                    for j in range(4):
                        for half in range(2):
                            pss = pbig.tile([128, 2, Q], f32, tag="bigp")
                            es = big.tile([128, 2, Q], bf16, tag="expS")
                            for gg in range(2):
                                g = 2 * half + gg
                                sl = slice(32 * g, 32 * g + 16)
                                nc.tensor.matmul(
                                    pss[:, gg, :],
                                    lhsT=k1t[sl, 128 * j:128 * (j + 1)],
                                    rhs=q1t_sb[sl, :], start=True, stop=False,
                                    tile_position=(32 * g, 0))
                                nc.tensor.matmul(
                                    pss[:, gg, :], lhsT=W["i128b"],
                                    rhs=mkt_t[:, j, :], start=False, stop=True)
                            nc.scalar.activation(es, pss, AF.Exp)
                            for gg in range(2):
                                g = 2 * half + gg
                                nc.tensor.matmul(
                                    psu[32 * g:32 * g + 32, :],
                                    lhsT=vauga[:, j, 32 * g:32 * g + 32]
                                    if pi == 0 else vaugb[:, j, 32 * g:32 * g + 32],
                                    rhs=es[:, gg, :],
                                    start=(j == 0), stop=(j == 3),
                                    tile_position=(0, 32 * g))
                    # copy U to sbuf; gather Z rows; normalize later
                    u_sb = wk.tile([128, Q], bf16, tag="usb")
                    nc.vector.tensor_copy(u_sb, psu)
                    usb[pi] = u_sb
                    pz = ptiny.tile([4, Q], f32, tag="tiny")
                    nc.tensor.matmul(pz, lhsT=W["g16"], rhs=u_sb)
                    nc.vector.tensor_copy(zsb[:, Q * pi:Q * (pi + 1)], pz)

                rinv = sm.tile([4, 1024], f32, tag="rinv")
                nc.vector.reciprocal_approx_fast(out=rinv, in_=zsb)
                rinvb = sm.tile([4, 1024], bf16, tag="rinvb")
                nc.vector.tensor_copy(rinvb, rinv)

                un = {}
                for pi in range(2):
                    pbc = pproj.tile([128, Q], f32, tag="proj")
                    nc.tensor.matmul(pbc, lhsT=W["e4"],
                                     rhs=rinvb[:, Q * pi:Q * (pi + 1)])
                    u_n = wk.tile([128, Q], bf16, tag=f"un{pi}")
                    nc.vector.tensor_tensor(u_n, usb[pi], pbc, OP.mult)
                    un[pi] = u_n

                # ---------- Q2 and logits ----------
                pq2 = pproj.tile([128, Q], f32, tag="proj")
                nc.tensor.matmul(pq2, lhsT=W["wouta"], rhs=un[0],
                                 start=True, stop=False)
                nc.tensor.matmul(pq2, lhsT=W["woutb"], rhs=un[1],
                                 start=False, stop=True)
                q2t = wk.tile([128, Q], bf16, tag="q2t")
                nc.vector.tensor_copy(q2t, pq2)

                tanh_sb = big.tile([128, C, N], f32, tag="tanh")
                for ii in range(2):
                    pl = pbig.tile([128, 2, N], f32, tag="bigp")
                    for i2 in range(2):
                        i = 2 * ii + i2
                        nc.tensor.matmul(
                            pl[:, i2, :],
                            lhsT=q2t[:, 128 * i:128 * (i + 1)], rhs=k2t)
                    nc.scalar.activation(
                        tanh_sb[:, 2 * ii:2 * ii + 2, :], pl, AF.Tanh)

                nc.vector.copy_predicated(tanh_sb, mknat_t, negt)

                # ---------- final log-softmax ----------
                sacc = sm.tile([128, 4], f32, tag="sacc")
                scratch = sm.tile([128, N], f32, tag="scratch")
                for i in range(4):
                    nc.scalar.activation(scratch, tanh_sb[:, i, :], AF.Exp,
                                         scale=10.0, accum_out=sacc[:, i:i + 1])
                # S [128(q'),4(i)] -> [4, 128] -> Z2 [4, 32]
                pst = ptiny.tile([4, 128], f32, tag="tiny")
                nc.tensor.transpose(pst, sacc, W["ident"])
                ssb = sm.tile([4, 128], f32, tag="ssb")
                nc.vector.tensor_copy(ssb, pst)
                z2 = sm.tile([4, 32], f32, tag="z2")
                nc.vector.tensor_reduce(
                    z2, ssb.rearrange("p (t c) -> p t c", c=4),
                    axis=mybir.AxisListType.X, op=OP.add)
                # ln via exponent bits + poly
                zi = z2.bitcast(i32)
                ei = sm.tile([4, 32], i32, tag="ei")
                nc.vector.tensor_scalar(ei, zi, 23, None,
                                        OP.logical_shift_right)
                ef = sm.tile([4, 32], f32, tag="ef")
                nc.vector.tensor_copy(ef, ei)
                mi = sm.tile([4, 32], i32, tag="mi")
                nc.vector.tensor_scalar(mi, zi, 0x7FFFFF, 0x3F800000,
                                        OP.bitwise_and, OP.bitwise_or)
                mf = mi.bitcast(f32)
                acc = sm.tile([4, 32], f32, tag="lnacc")
                nc.vector.tensor_scalar(acc, mf, LN_COEF[7], LN_COEF[6],
                                        OP.mult, OP.add)
                for k in range(5, -1, -1):
                    nc.vector.tensor_tensor(acc, acc, mf, OP.mult)
                    nc.vector.tensor_scalar_add(acc, acc, LN_COEF[k])
                # lnZ2 = acc + (ef - 127)*LN2
                nc.vector.tensor_scalar(ef, ef, LN2, -127.0 * LN2,
                                        OP.mult, OP.add)
                nc.vector.tensor_tensor(acc, acc, ef, OP.add)
                # transpose [4,32] -> [32,4]; broadcast to [128,4]
                pzt = ptiny.tile([32, 4], f32, tag="tiny")
                nc.tensor.transpose(pzt, acc, W["ident"][:4, :4])
                lzt = sm.tile([32, 4], f32, tag="lzt")
                nc.vector.tensor_copy(lzt, pzt)
                pbias = ptiny.tile([128, 4], f32, tag="tiny")
                nc.tensor.matmul(pbias, lhsT=W["p432"], rhs=lzt)
                bias = sm.tile([128, 4], f32, tag="bias")
                nc.vector.tensor_copy(bias, pbias)

                out_sb = big.tile([128, C, N], f32, tag="outsb")
                for i in range(4):
                    nc.vector.tensor_scalar(
                        out_sb[:, i, :], tanh_sb[:, i, :], 10.0,
                        bias[:, i:i + 1], OP.mult, OP.subtract)
                nc.sync.dma_start(out=out[:, b, :, :], in_=out_sb)

    nc.compile()
    return nc


_CACHED = None


def _get_nc():
    global _CACHED
    if _CACHED is None:
        _CACHED = build_kernel()
    return _CACHED


def kernel(**inputs):
    from concourse.bass_utils import run_bass_kernel_spmd

    core_ins = _host_prep(inputs)
    nc = _get_nc()
    res = run_bass_kernel_spmd(nc, core_ins, core_ids=list(range(NCORES)))
    outs = [_unscramble(r["out"]) for r in res.results]   # each [T, NB, 2048]
    return np.concatenate(outs, axis=1)                   # [T, B, 2048]


def _unscramble(dev):
    """Device [128 q'=(t',c), nb, 4 i, 512 n] -> [T, nb, C*N] with t=32i+t'."""
    nb = dev.shape[1]
    return (dev.reshape(32, C, nb, 4, N)
            .transpose(3, 0, 2, 1, 4)
            .reshape(T, nb, C * N))
